# revision 6
# baseline (speedup 1.0000x reference)
"""DeepSigNet Trainium2 kernel (8-core data-parallel, bf16 datapath).

Math (per batch element, matching the reference):
  path = tanh(conv1d(x[:64], w, k=3, pad=1) + b).T          # [L=512, 64]
  dd[t] = path[t+1] - path[t]
  st[m, j] = sum_t dd[t, m] * path[t, j]  (+ p0 (x) p511 via a patched
  dd row; the ones-column of the rhs telescopes sum(dd) = lvl1).
  Only the antisymmetric part of st feeds the MLP, so fc1 weights are
  host-antisymmetrized and consume the full st matrix directly.

Device layout (per core, 16 batch elems):
  FT [128, 576] bf16: 36 K-tiles of 16 columns (one col per batch elem).
    tiles 0..31: FT[p<64, 16t+b] = st_b[p, t]; FT[p>=64] = st_b[p-64, 32+t]
    tile 32: p<64 lvl1; p>=64 static chans 65..128 (host aux)
    tiles 33, 34: static chans 129..384 (host aux)
    tile 35: p0 pooled max (device); p1 const-1 (fc1 bias); p2..64
      static chans 385..447 (host aux)
  Everything fp32 lives only in PSUM accumulators, h1/h2 staging and the
  final output; all matmul operands are bf16 (1 cycle/row on PE).
"""

import os
import numpy as np

B, C_IN, C_OUT, L = 128, 64, 64, 512
POST, HID, OUT_DIM = 384, 1024, 128
NCORES = 8
BPC = B // NCORES   # 16
NT1 = 36            # fc1 K-tiles
D1 = NT1 * 128      # 4608 padded fc1 input dim
XW = 514            # per-elem column block in xg (512 + 2 pad cols)

GE = int(os.environ.get("DSN_GE", "4"))          # front-end group size
W1CHUNK = int(os.environ.get("DSN_W1CHUNK", "4"))  # fc1 K-tiles per DMA

_prog_cache = {}
_host_cache = {}


def _build_nc():
    key = ("nc", GE, W1CHUNK)
    if key in _prog_cache:
        return _prog_cache[key]

    import concourse.bass as bass
    import concourse.tile as tile
    from concourse import bacc, mybir

    f32 = mybir.dt.float32
    bf16 = mybir.dt.bfloat16
    TANH = mybir.ActivationFunctionType.Tanh

    nc = bacc.Bacc(None, target_bir_lowering=False, debug=False)

    xcv_d = nc.dram_tensor("xcv", [BPC, C_IN, L], bf16, kind="ExternalInput")
    # pooled-max channel (f32) + a 16x16 identity for PE transposes
    xpi_d = nc.dram_tensor("xpi", [BPC, L + 16], f32, kind="ExternalInput")
    # wkb: cols 0:192 conv taps (+bias row 64), 192:208 ocst ones-cols,
    # col 208 ones16 row
    wkb_d = nc.dram_tensor("wkb", [128, 224], bf16, kind="ExternalInput")
    # aux: static feature tiles 32..35 content ([128, 4*16])
    aux_d = nc.dram_tensor("aux", [128, 4 * BPC], bf16, kind="ExternalInput")
    b23_d = nc.dram_tensor("b23", [1, HID + OUT_DIM], bf16, kind="ExternalInput")
    w1_d = nc.dram_tensor("w1t", [D1, HID], bf16, kind="ExternalInput")
    w2_d = nc.dram_tensor("w2t", [HID, HID], bf16, kind="ExternalInput")
    w3_d = nc.dram_tensor("w3t", [HID, OUT_DIM], bf16, kind="ExternalInput")
    out_d = nc.dram_tensor("out", [BPC, OUT_DIM], f32, kind="ExternalOutput")

    xa = xcv_d.ap()
    outa = out_d.ap()
    nchunk = NT1 // W1CHUNK

    with tile.TileContext(nc) as tc:
        with (
            tc.tile_pool(name="const", bufs=1) as constp,
            tc.tile_pool(name="big", bufs=1) as bigp,
            tc.tile_pool(name="cvps", bufs=3, space="PSUM") as cvpsp,
            tc.tile_pool(name="smallps", bufs=3, space="PSUM") as smallps,
            tc.tile_pool(name="xg", bufs=2) as xgp,
            tc.tile_pool(name="ptg", bufs=2) as ptgp,
            tc.tile_pool(name="ptshg", bufs=2) as ptshgp,
            tc.tile_pool(name="ddg", bufs=2) as ddgp,
            tc.tile_pool(name="prow", bufs=2) as prowp,
            tc.tile_pool(name="wstream", bufs=nchunk) as wsp,
            tc.tile_pool(name="wstream2", bufs=2) as wsp2,
            tc.tile_pool(name="mlpps", bufs=2, space="PSUM") as mlpps,
            tc.tile_pool(name="act", bufs=1) as actp,
        ):
            # --- weight stream: issue every weight DMA up front (sync
            # queue) so the bus stays saturated behind the front-end ---
            w1tiles = []
            for ck in range(nchunk):
                w1s = wsp.tile([128, W1CHUNK * HID], bf16, tag="ws")
                nc.sync.dma_start(
                    w1s[:].rearrange("p (t h) -> p t h", h=HID),
                    w1_d.ap()[128 * W1CHUNK * ck:128 * W1CHUNK * (ck + 1), :]
                    .rearrange("(t p) h -> p t h", p=128))
                w1tiles.append(w1s)
            w2tiles = []
            for ck in range(2):
                w2s = wsp2.tile([128, 4 * HID], bf16, name=f"w2s{ck}")
                nc.sync.dma_start(
                    w2s[:].rearrange("p (t h) -> p t h", h=HID),
                    w2_d.ap()[512 * ck:512 * (ck + 1), :]
                    .rearrange("(t p) h -> p t h", p=128))
                w2tiles.append(w2s)
            w3s = actp.tile([128, HID], bf16)
            nc.sync.dma_start(
                w3s[:].rearrange("p (t o) -> p t o", o=OUT_DIM),
                w3_d.ap()[:, :].rearrange("(t p) o -> p t o", p=128))

            # --- constants (vector queue; tiny) ---
            wkb = constp.tile([128, 224], bf16)
            nc.gpsimd.dma_start(wkb[:], wkb_d.ap()[:, :])
            wcv = wkb[:, 0:192]
            ocst = wkb[:, 192:208]
            ones16 = wkb[0:1, 208:224]
            b23 = constp.tile([1, HID + OUT_DIM], bf16)
            nc.gpsimd.dma_start(b23[:], b23_d.ap()[:, :])
            b2s = b23[:, 0:HID]
            b3s = b23[:, HID:HID + OUT_DIM]
            xpi = constp.tile([BPC, L + 16], f32)
            nc.scalar.dma_start(xpi[:], xpi_d.ap()[:, :])
            idn16 = xpi[:, L:L + 16]

            # --- persistent feature tensor ---
            ft = bigp.tile([128, NT1 * BPC], bf16)
            ftr = ft[:].rearrange("p (t c) -> p t c", c=BPC)
            # static/aux tiles 32..35 straight from the host
            nc.gpsimd.dma_start(ft[:, 32 * BPC:36 * BPC], aux_d.ap()[:, :])

            # ===== front-end, pipelined in groups of GE elems =====
            ocstr = ocst[:].rearrange("p (e b) -> p e b", b=4)
            for g in range(BPC // GE):
                e0 = GE * g
                # xg: 64 conv chans + a ones row (bias); per elem 514 cols
                # (col 0 / col 513 zero-pad for the k=3 conv)
                xg = xgp.tile([C_IN + 1, GE * XW], bf16)
                xgr = xg[:].rearrange("p (e w) -> p e w", w=XW)
                (nc.sync if g < 2 else nc.scalar).dma_start(
                    xgr[0:64, :, 1:513],
                    xa[e0:e0 + GE, :, :].rearrange("e c l -> c e l"))
                nc.gpsimd.memset(xgr[0:64, :, 0:1], 0.0)
                nc.gpsimd.memset(xgr[0:64, :, 513:514], 0.0)
                nc.gpsimd.memset(xg[64:65, :], 1.0)

                # conv via 3 tap-matmuls (K=65 incl bias row) per L-block
                ptg = ptgp.tile([128, GE * 260], bf16)
                pt4 = ptg[:].rearrange("p (e b c) -> p e b c", b=4, c=65)
                nc.vector.tensor_copy(pt4[:, :, :, 64], ocstr[:, 0:GE, :])
                for i in range(GE):
                    xo = XW * i
                    cv = cvpsp.tile([128, 256], f32)
                    for lt in range(4):
                        for k in range(3):
                            nc.tensor.matmul(
                                cv[:, 64 * lt:64 * lt + 64],
                                xg[0:65, xo + 128 * lt + k:
                                   xo + 128 * lt + k + 128],
                                wcv[0:65, 64 * k:64 * k + 64],
                                start=(k == 0), stop=(k == 2))
                    nc.scalar.activation(
                        pt4[:, i, :, 0:64],
                        cv[:].rearrange("p (b c) -> p b c", c=64), TANH)

                # shifted path (partition shifts go through DMA); row 127
                # of blocks 0..2 = next block's row 0, block 3 = p511
                # (so dd row 511 is 0); the +p0 (x) p511 correction enters
                # as a rank-1 matmul from prowg
                ptshg = ptshgp.tile([128, GE * 260], bf16)
                psh4 = ptshg[:].rearrange("p (e b c) -> p e b c", b=4, c=65)
                nc.scalar.dma_start(ptshg[0:127, :], ptg[1:128, :])
                nc.scalar.dma_start(
                    psh4[127:128, :, 0:3, :], pt4[0:1, :, 1:4, :])
                nc.scalar.dma_start(
                    psh4[127:128, :, 3, 0:65], pt4[127:128, :, 3, 0:65])
                prowg = prowp.tile([1, GE * 64], bf16)
                nc.scalar.dma_start(
                    prowg[:].rearrange("p (e c) -> p e c", c=64),
                    pt4[127:128, :, 3, 0:64])
                ddg = ddgp.tile([128, GE * 260], bf16)
                nc.vector.tensor_sub(ddg[:, :], ptshg[:, :], ptg[:, :])

                # log-signature st per elem
                for i in range(GE):
                    po = 260 * i
                    st = smallps.tile([128, 65], f32, tag="sm", name="st")
                    for t in range(4):
                        nc.tensor.matmul(
                            st[0:64, :],
                            ddg[:, po + 65 * t:po + 65 * t + 64],
                            ptg[:, po + 65 * t:po + 65 * t + 65],
                            start=(t == 0), stop=(t == 3))
                    nc.tensor.matmul(
                        st[0:64, 0:64], ptg[0:1, po:po + 64],
                        prowg[0:1, 64 * i:64 * i + 64],
                        start=False, stop=True, skip_group_check=True)
                    e = e0 + i
                    nc.vector.tensor_copy(ftr[0:64, 0:32, e], st[0:64, 0:32])
                    nc.vector.tensor_copy(ftr[64:128, 0:32, e], st[0:64, 32:64])
                    nc.vector.tensor_copy(
                        ft[0:64, 32 * BPC + e:32 * BPC + e + 1], st[0:64, 64:65])

            # ======== pooled max (channel C_IN) ========
            pxm = actp.tile([BPC, 1], f32)
            nc.vector.reduce_max(pxm[:, :], xpi[:, 0:L],
                                 axis=bass.mybir.AxisListType.X)
            pxt = smallps.tile([128, 65], f32, tag="sm", name="pxt")
            nc.tensor.transpose(pxt[0:1, 0:BPC], pxm[:, :], idn16[0:BPC, 0:BPC])
            nc.vector.tensor_copy(ft[0:1, 35 * BPC:36 * BPC], pxt[0:1, 0:BPC])

            # ======================= MLP =======================
            # fc1: H1[b, h] = FT.T @ W1T
            h1ps = [mlpps.tile([BPC, 512], f32, tag="hps", name=f"h1ps{i}")
                    for i in range(2)]
            h1 = actp.tile([BPC, HID], f32)
            h1t = actp.tile([128, 128], bf16)
            for ck in range(nchunk):
                for t in range(W1CHUNK):
                    kt = W1CHUNK * ck + t
                    for nt in range(2):
                        nc.tensor.matmul(
                            h1ps[nt][:, :],
                            ftr[:, kt, :],
                            w1tiles[ck][:, HID * t + 512 * nt:
                                        HID * t + 512 * nt + 512],
                            start=(kt == 0), stop=(kt == NT1 - 1))
            for nt in range(2):
                nc.vector.tensor_relu(h1[:, 512 * nt:512 * nt + 512],
                                      h1ps[nt][:, :])
                for i in range(4 * nt, 4 * nt + 4):
                    tp = smallps.tile([128, 65], f32, tag="sm", name="tp")
                    nc.tensor.transpose(
                        tp[:, 0:BPC], h1[:, 128 * i:128 * i + 128],
                        idn16[0:BPC, 0:BPC])
                    nc.vector.tensor_copy(
                        h1t[:, 16 * i:16 * i + 16], tp[:, 0:BPC])

            # fc2
            h2ps = [mlpps.tile([BPC, 512], f32, tag="hps", name=f"h2ps{i}")
                    for i in range(2)]
            for ck in range(2):
                for t in range(4):
                    kt = 4 * ck + t
                    for nt in range(2):
                        nc.tensor.matmul(
                            h2ps[nt][:, :],
                            h1t[:, 16 * kt:16 * kt + 16],
                            w2tiles[ck][:, HID * t + 512 * nt:
                                        512 * nt + HID * t + 512],
                            start=(kt == 0), stop=False)
            h2 = actp.tile([BPC, HID], f32)
            h2t = actp.tile([128, 128], bf16)
            for nt in range(2):
                nc.tensor.matmul(
                    h2ps[nt][:, :], ones16[:, :],
                    b2s[0:1, 512 * nt:512 * nt + 512],
                    start=False, stop=True)
                nc.vector.tensor_relu(h2[:, 512 * nt:512 * nt + 512],
                                      h2ps[nt][:, :])
                for i in range(4 * nt, 4 * nt + 4):
                    tp = smallps.tile([128, 65], f32, tag="sm", name="tp")
                    nc.tensor.transpose(
                        tp[:, 0:BPC], h2[:, 128 * i:128 * i + 128],
                        idn16[0:BPC, 0:BPC])
                    nc.vector.tensor_copy(
                        h2t[:, 16 * i:16 * i + 16], tp[:, 0:BPC])

            # fc3
            ops = mlpps.tile([BPC, 512], f32, tag="hps")
            for kt in range(8):
                nc.tensor.matmul(
                    ops[:, 0:OUT_DIM],
                    h2t[:, 16 * kt:16 * kt + 16],
                    w3s[:, OUT_DIM * kt:OUT_DIM * kt + OUT_DIM],
                    start=(kt == 0), stop=False)
            nc.tensor.matmul(
                ops[:, 0:OUT_DIM], ones16[:, :], b3s, start=False, stop=True)
            outsb = actp.tile([BPC, OUT_DIM], f32)
            nc.vector.tensor_copy(outsb[:, :], ops[:, 0:OUT_DIM])
            nc.scalar.dma_start(outa[:, :], outsb[:, :])

    nc.compile()
    _prog_cache[key] = nc
    return nc


def _host_weights(conv_w, conv_b, fc1_w, fc1_b, fc2_w, fc2_b, fc3_w, fc3_b):
    import ml_dtypes
    f = np.float32
    bf = ml_dtypes.bfloat16
    conv_w = np.asarray(conv_w, f)
    fc1_w = np.asarray(fc1_w, f)

    wkb = np.zeros((128, 224), f)
    # conv taps: wcv[i, 64k + o] = conv_w[o, i, k]; row 64 = bias (tap 0)
    wt = conv_w.transpose(1, 2, 0)                 # [i, k, o]
    wkb[0:64, 0:192] = wt.reshape(64, 192)
    wkb[64, 0:64] = np.asarray(conv_b, f)
    # ocst ones-columns: per (elem-in-group, block); zero at (127, blk 3)
    wkb[:, 192:208] = 1.0
    wkb[127, 192 + 3::4] = 0.0
    wkb[0, 208:224] = 1.0                          # ones16 row

    # fc1: antisymmetrize the Lyndon-word weights onto the full st matrix
    wfull = np.zeros((HID, 64, 64), f)
    iu, ju = np.triu_indices(64, 1)
    wtri = fc1_w[:, 64:2080]
    wfull[:, iu, ju] = 0.5 * wtri
    wfull[:, ju, iu] = -0.5 * wtri

    w1t = np.zeros((D1, HID), f)
    for t in range(32):
        w1t[128 * t:128 * t + 64, :] = wfull[:, t, :].T
        w1t[128 * t + 64:128 * t + 128, :] = wfull[:, 32 + t, :].T
    # tile 32: lvl1 (p<64), static chans 65..128 (p>=64)
    w1t[4096:4160, :] = fc1_w[:, 0:64].T
    w1t[4160:4224, :] = fc1_w[:, 2081:2145].T
    # tiles 33, 34: static chans 129..384
    w1t[4224:4352, :] = fc1_w[:, 2145:2273].T
    w1t[4352:4480, :] = fc1_w[:, 2273:2401].T
    # tile 35: p0 pooled, p1 const-1 -> fc1 bias, p2..64 static 385..447
    w1t[4480, :] = fc1_w[:, 2080]
    w1t[4481, :] = np.asarray(fc1_b, f)
    w1t[4482:4545, :] = fc1_w[:, 2401:2464].T

    w2t = np.ascontiguousarray(np.asarray(fc2_w, f).T)
    w3t = np.ascontiguousarray(np.asarray(fc3_w, f).T)
    b23 = np.concatenate(
        [np.asarray(fc2_b, f), np.asarray(fc3_b, f)])[None, :]
    return dict(wkb=wkb.astype(bf), b23=b23.astype(bf),
                w1t=w1t.astype(bf), w2t=w2t.astype(bf), w3t=w3t.astype(bf))


def make_in_maps(x, conv_w, conv_b, fc1_w, fc1_b, fc2_w, fc2_b, fc3_w, fc3_b):
    import ml_dtypes
    bf = ml_dtypes.bfloat16
    wkey = id(fc1_w)
    if _host_cache.get("wkey") != wkey:
        _host_cache["shared"] = _host_weights(
            conv_w, conv_b, fc1_w, fc1_b, fc2_w, fc2_b, fc3_w, fc3_b)
        _host_cache["wkey"] = wkey
    shared = _host_cache["shared"]
    x = np.asarray(x, np.float32)
    eye16 = np.eye(BPC, dtype=np.float32)
    in_maps = []
    for c in range(NCORES):
        xc = x[BPC * c:BPC * (c + 1)]
        m = dict(shared)
        m["xcv"] = np.ascontiguousarray(xc[:, 0:C_IN, :]).astype(bf)
        m["xpi"] = np.concatenate([xc[:, C_IN, :], eye16], axis=1)
        aux = np.zeros((128, 4 * BPC), np.float32)
        aux[64:128, 0:BPC] = xc[:, 65:129, 0].T          # tile 32 (p>=64)
        aux[:, BPC:2 * BPC] = xc[:, 129:257, 0].T        # tile 33
        aux[:, 2 * BPC:3 * BPC] = xc[:, 257:385, 0].T    # tile 34
        aux[1, 3 * BPC:4 * BPC] = 1.0                    # fc1 bias row
        aux[2:65, 3 * BPC:4 * BPC] = xc[:, 385:448, 0].T  # tile 35 statics
        m["aux"] = aux.astype(bf)
        in_maps.append(m)
    return in_maps


def kernel(x, conv_w, conv_b, fc1_w, fc1_b, fc2_w, fc2_b, fc3_w, fc3_b):
    from concourse.bass_utils import run_bass_kernel_spmd

    nc = _build_nc()
    in_maps = make_in_maps(x, conv_w, conv_b, fc1_w, fc1_b, fc2_w, fc2_b,
                           fc3_w, fc3_b)
    res = run_bass_kernel_spmd(nc, in_maps, list(range(NCORES)))
    out = np.concatenate([res.results[c]["out"] for c in range(NCORES)], axis=0)
    return out.astype(np.float32)


# revision 25
# speedup vs baseline: 1.0510x; 1.0510x over previous
"""DeepSigNet Trainium2 kernel (8-core data-parallel, bf16 datapath).

Math (per batch element, matching the reference):
  path = tanh(conv1d(x[:64], w, k=3, pad=1) + b).T          # [L=512, 64]
  dd[t] = path[t+1] - path[t]
  st[m, j] = sum_t dd[t, m] * path[t, j]  (+ p0 (x) p511 via a rank-1
  matmul; the ones-column of the rhs telescopes sum(dd) = lvl1).
  Only the antisymmetric part of st feeds the MLP, so fc1 weights are
  host-antisymmetrized and consume the full st matrix directly.

Device layout (per core, 16 batch elems):
  FT [128, 576] bf16: 36 K-tiles of 16 columns (one col per batch elem).
    tiles 0..31: FT[p<64, 16t+b] = st_b[p, t]; FT[p>=64] = st_b[p-64, 32+t]
    tile 32: p<64 lvl1; p>=64 static chans 65..128 (host aux)
    tiles 33, 34: static chans 129..384 (host aux)
    tile 35: p0 pooled max (device); p1 const-1 (fc1 bias); p2..64
      static chans 385..447 (host aux)
  All matmul operands are bf16 (1 cycle/row on PE); fp32 only in PSUM
  accumulators, h1/h2 staging and the final output.

Schedule: x loads head the DMA queue, the weight stream saturates the
bus behind them; the front-end is software-pipelined (conv of group g+1
issues before the signature matmuls of group g, since engines execute
strictly in order); fc1 consumes weight chunks as they arrive; fc2/fc3
interleave with the h1/h2 transpose halves.
"""

import os
import numpy as np

B, C_IN, C_OUT, L = 128, 64, 64, 512
POST, HID, OUT_DIM = 384, 1024, 128
NCORES = 8
BPC = B // NCORES   # 16
NT1 = 28            # fc1 K-tiles (24 signature + 4 aux)
D1 = NT1 * 128      # 3584 padded fc1 input dim
XW = 514            # per-elem column block in xg (512 + 2 pad cols)

GE = int(os.environ.get("DSN_GE", "4"))          # front-end group size
W1CHUNK = 4
# fc1 K-tiles per weight DMA; a small final chunk minimizes the work
# trailing the weight stream
W1CHUNKS = [4] * 6 + [3, 1]
# fc1 kt index -> device ft tile (aux tiles stream first)
KT2FT = [25, 26, 27, 24] + list(range(24))

_prog_cache = {}
_host_cache = {}


def _build_nc():
    key = ("nc", GE, W1CHUNK)
    if key in _prog_cache:
        return _prog_cache[key]

    import concourse.bass as bass
    import concourse.tile as tile
    from concourse import bacc, mybir

    f32 = mybir.dt.float32
    bf16 = mybir.dt.bfloat16
    TANH = mybir.ActivationFunctionType.Tanh
    COPY = mybir.ActivationFunctionType.Copy

    nc = bacc.Bacc(None, target_bir_lowering=False, debug=False)

    xcv_d = nc.dram_tensor("xcv", [BPC, C_IN + 1, XW], bf16,
                           kind="ExternalInput")
    # pooled-max channel (f32) + a 16x16 identity for PE transposes
    xpi_d = nc.dram_tensor("xpi", [BPC, L + 16], f32, kind="ExternalInput")
    # wkb: cols 0:192 conv taps (+bias row 64), 192:208 ocst ones-cols,
    # cols 208:224 ones16 row, 224:352 shift-by-one identity, 352:480 e127
    wkb_d = nc.dram_tensor("wkb", [128, 480], bf16, kind="ExternalInput")
    # 64x64 f32 identity for the per-elem st transpose
    c64_d = nc.dram_tensor("c64", [64, 64], f32, kind="ExternalInput")
    # aux: static feature tiles 24..27 content ([128, 4*16])
    aux_d = nc.dram_tensor("aux", [128, 4 * BPC], bf16, kind="ExternalInput")
    b23_d = nc.dram_tensor("b23", [1, HID + OUT_DIM], bf16,
                           kind="ExternalInput")
    w1_d = nc.dram_tensor("w1t", [D1, HID], bf16, kind="ExternalInput")
    w2_d = nc.dram_tensor("w2t", [HID, HID], bf16, kind="ExternalInput")
    # w3t pre-packed on host into the on-chip layout [128, 8*128]
    w3_d = nc.dram_tensor("w3t", [128, HID], bf16, kind="ExternalInput")
    out_d = nc.dram_tensor("out", [BPC, OUT_DIM], f32, kind="ExternalOutput")

    xa = xcv_d.ap()
    outa = out_d.ap()
    nchunk = len(W1CHUNKS)
    ngroup = BPC // GE
    cof = [sum(W1CHUNKS[:i]) for i in range(nchunk + 1)]  # chunk offsets
    kt2ck = {kt: (ck, kt - cof[ck]) for ck in range(nchunk)
             for kt in range(cof[ck], cof[ck + 1])}

    with tile.TileContext(nc) as tc:
        with (
            tc.tile_pool(name="const", bufs=1) as constp,
            tc.tile_pool(name="big", bufs=1) as bigp,
            tc.tile_pool(name="cvps", bufs=2, space="PSUM") as cvpsp,
            tc.tile_pool(name="pshps", bufs=2, space="PSUM") as pshpsp,
            tc.tile_pool(name="smallps", bufs=2, space="PSUM") as smallps,
            tc.tile_pool(name="xg", bufs=ngroup) as xgp,
            tc.tile_pool(name="ptg", bufs=4) as ptgp,
            tc.tile_pool(name="ddg", bufs=4) as ddgp,
            tc.tile_pool(name="sta", bufs=2) as stap,
            tc.tile_pool(name="tt", bufs=2) as ttp,
            tc.tile_pool(name="wstream", bufs=6) as wsp,
            tc.tile_pool(name="wstreamtl", bufs=1) as wsptl,
            tc.tile_pool(name="wstream2", bufs=2) as wsp2,
            tc.tile_pool(name="mlpps", bufs=2, space="PSUM") as mlpps,
            tc.tile_pool(name="act", bufs=1) as actp,
        ):
            # --- front-end x loads first (sync queue head), then the
            # weight stream saturates the bus behind them ---
            xgs = []
            for g in range(ngroup):
                xg = xgp.tile([C_IN + 1, GE * XW], bf16, name=f"xg{g}")
                nc.sync.dma_start(
                    xg[:].rearrange("p (e w) -> p e w", w=XW),
                    xa[GE * g:GE * (g + 1), :, :].rearrange("e c l -> c e l"))
                xgs.append(xg)

            w1tiles = []
            for ck in range(nchunk):
                pool = wsp if W1CHUNKS[ck] == 4 else wsptl
                w1s = pool.tile([128, W1CHUNKS[ck] * HID], bf16,
                                tag=f"ws{W1CHUNKS[ck]}", name=f"w1s{ck}")
                nc.sync.dma_start(
                    w1s[:].rearrange("p (t h) -> p t h", h=HID),
                    w1_d.ap()[128 * cof[ck]:128 * cof[ck + 1], :]
                    .rearrange("(t p) h -> p t h", p=128))
                w1tiles.append(w1s)
            w2tiles = []
            for ck in range(2):
                w2s = wsp2.tile([128, 4 * HID], bf16, name=f"w2s{ck}")
                nc.sync.dma_start(
                    w2s[:].rearrange("p (t h) -> p t h", h=HID),
                    w2_d.ap()[512 * ck:512 * (ck + 1), :]
                    .rearrange("(t p) h -> p t h", p=128))
                w2tiles.append(w2s)
            w3s = actp.tile([128, HID], bf16)
            nc.sync.dma_start(w3s[:], w3_d.ap()[:, :])

            # --- constants (SWDGE; tiny, off the critical queues) ---
            wkb = constp.tile([128, 480], bf16)
            nc.gpsimd.dma_start(wkb[:], wkb_d.ap()[:, :])
            wcv = wkb[:, 0:192]
            ocst = wkb[:, 192:208]
            ones16 = wkb[0:1, 208:224]
            shm = wkb[:, 224:352]
            e127 = wkb[0:1, 352:480]
            b23 = constp.tile([1, HID + OUT_DIM], bf16)
            nc.gpsimd.dma_start(b23[:], b23_d.ap()[:, :])
            b2s = b23[:, 0:HID]
            b3s = b23[:, HID:HID + OUT_DIM]
            xpi = constp.tile([BPC, L + 16], f32)
            nc.scalar.dma_start(xpi[:], xpi_d.ap()[:, :])
            idn16 = xpi[:, L:L + 16]
            idn64 = constp.tile([64, 64], f32)
            nc.scalar.dma_start(idn64[:], c64_d.ap()[:, :])

            # --- persistent feature tensor ---
            ft = bigp.tile([128, NT1 * BPC], bf16)
            ftr = ft[:].rearrange("p (t c) -> p t c", c=BPC)
            # static/aux tiles 32..35 straight from the host
            nc.gpsimd.dma_start(ft[:, 24 * BPC:28 * BPC], aux_d.ap()[:, :])

            # ======== pooled max (channel C_IN) ========
            pxm = actp.tile([BPC, 1], f32)
            nc.vector.reduce_max(pxm[:, :], xpi[:, 0:L],
                                 axis=bass.mybir.AxisListType.X)
            pxt = smallps.tile([128, 65], f32, tag="sm", name="pxt")
            nc.tensor.transpose(pxt[0:1, 0:BPC], pxm[:, :], idn16[0:BPC, 0:BPC])
            nc.vector.tensor_copy(ft[0:1, 27 * BPC:28 * BPC], pxt[0:1, 0:BPC])

            h1ps = [mlpps.tile([BPC, 512], f32, tag="hps", name=f"h1ps{i}")
                    for i in range(2)]

            def fc1_mm(kt, nt):
                ck, t = kt2ck[kt]
                nc.tensor.matmul(
                    h1ps[nt][:, :],
                    ftr[:, KT2FT[kt], :],
                    w1tiles[ck][:, HID * t + 512 * nt:
                                HID * t + 512 * nt + 512],
                    start=(kt == 0), stop=(kt == NT1 - 1))

            # ===== front-end, phase-major so each engine streams =====
            ocstr = ocst[:].rearrange("p (e b) -> p e b", b=4)
            gstate = {}

            def conv_phase(g):
                xg = xgs[g]
                ptg = ptgp.tile([128, GE * 260], bf16, tag="pt")
                pt4 = ptg[:].rearrange("p (e b c) -> p e b c", b=4, c=65)
                nc.vector.tensor_copy(pt4[:, :, :, 64], ocstr[:, 0:GE, :])
                for i in range(GE):
                    xo = XW * i
                    if i % 2 == 0:
                        cvp = cvpsp.tile([128, 512], f32)
                    co = 256 * (i % 2)
                    for lt in range(4):
                        for k in range(3):
                            nc.tensor.matmul(
                                cvp[:, co + 64 * lt:co + 64 * lt + 64],
                                xg[0:65, xo + 128 * lt + k:
                                   xo + 128 * lt + k + 128],
                                wcv[0:65, 64 * k:64 * k + 64],
                                start=(k == 0), stop=(k == 2))
                    if i % 2 == 1:
                        nc.scalar.activation(
                            pt4[:, i - 1:i + 1, :, 0:64],
                            cvp[:].rearrange("p (e b c) -> p e b c", c=64,
                                             b=4), TANH)

                gstate[g] = ptg
            def psh_phase(g):
                ptg = gstate[g]
                pt4 = ptg[:].rearrange("p (e b c) -> p e b c", b=4, c=65)
                ddg = ddgp.tile([128, GE * 260], bf16, tag="dd",
                                name=f"dd{g}")
                for i in range(GE):
                    po = 260 * i
                    # shifted path via PE: shm is a shift-by-one identity so
                    # psh[m] = path[m+1] within each 128-block; row 127 of
                    # blocks 0..2 = next block's row 0 (e127 outer products);
                    # row 127 of block 3 = p0, so dd's virtual row 511 is
                    # p0 - p511, contributing p0 (x) p511 modulo a symmetric
                    # matrix that the antisymmetrized fc1 weights kill
                    psh = pshpsp.tile([128, 260], f32, tag="ps", name="psh")
                    pshb = psh[:].rearrange("p (b c) -> p b c", c=65)
                    nc.tensor.matmul(psh[:, :], shm, ptg[:, po:po + 260],
                                     start=True, stop=False)
                    for b in range(3):
                        nc.tensor.matmul(pshb[:, b, 0:64], e127,
                                         pt4[0:1, i, 1 + b, 0:64],
                                         start=False, stop=False)
                    nc.tensor.matmul(pshb[:, 3, 0:64], e127,
                                     pt4[0:1, i, 0, 0:64],
                                     start=False, stop=True)
                    nc.vector.tensor_sub(ddg[:, po:po + 260], psh[:, :],
                                         ptg[:, po:po + 260])
                gstate[g] = (ptg, ddg)

            def sig_b(g):
                ptg, ddg = gstate.pop(g)
                # signature matrices for the whole group in one PSUM tile
                stg = smallps.tile([64, GE * 65], f32, tag="sm", name="stg")
                stgr = stg[:].rearrange("p (e c) -> p e c", c=65)
                for i in range(GE):
                    po = 260 * i
                    for t in range(4):
                        nc.tensor.matmul(
                            stg[:, 65 * i:65 * i + 65],
                            ddg[:, po + 65 * t:po + 65 * t + 64],
                            ptg[:, po + 65 * t:po + 65 * t + 65],
                            start=(t == 0), stop=(t == 3))
                e0 = GE * g
                stA = stap.tile([64, GE * 64], f32, tag="sta")
                nc.vector.tensor_copy(
                    stA[:].rearrange("p (e c) -> p e c", c=64),
                    stgr[:, :, 0:64])
                nc.scalar.activation(
                    ft[0:64, 24 * BPC + e0:24 * BPC + e0 + GE],
                    stgr[:, :, 64], COPY)
                cstate[g] = stA

            def sig_c(g):
                stA = cstate.pop(g)
                e0 = GE * g
                # T = st - st^T is antisymmetric: its columns 0:32 cover
                # every index pair except those entirely in 32:64, which
                # the [32:64]^2 block supplies (tiles 16..23, 4 col-slots
                # of 32 partitions); host weights carry the signs.
                stT = pshpsp.tile([64, GE * 64], f32, tag="ps", name="stT")
                for i in range(GE):
                    nc.tensor.transpose(stT[:, 64 * i:64 * i + 64],
                                        stA[:, 64 * i:64 * i + 64], idn64[:])
                tt = ttp.tile([64, GE * 64], bf16, tag="tt")
                nc.vector.tensor_sub(tt[:, :], stA[:, :], stT[:, :])
                ttr = tt[:].rearrange("p (e c) -> p c e", c=64)
                nc.scalar.activation(ftr[0:64, 0:16, e0:e0 + GE],
                                     ttr[:, 0:16, :], COPY)
                nc.scalar.activation(ftr[64:128, 0:16, e0:e0 + GE],
                                     ttr[:, 16:32, :], COPY)
                for sl in range(4):
                    eng = nc.vector if sl < 2 else nc.scalar
                    if sl < 2:
                        nc.vector.tensor_copy(
                            ftr[32 * sl:32 * sl + 32, 16:24, e0:e0 + GE],
                            ttr[32:64, 32 + 8 * sl:40 + 8 * sl, :])
                    else:
                        nc.scalar.activation(
                            ftr[32 * sl:32 * sl + 32, 16:24, e0:e0 + GE],
                            ttr[32:64, 32 + 8 * sl:40 + 8 * sl, :], COPY)
            cstate = {}
            for g in range(ngroup):
                conv_phase(g)
            for g in range(ngroup):
                psh_phase(g)
            # B/C interleaved so C(g)'s transposes never stall the PE;
            # aux-tile fc1 matmuls (static features, ready early) fill the
            # PE stall gaps between phases
            fc1_mm(0, 0)
            fc1_mm(0, 1)
            sig_b(0)
            fc1_mm(1, 0)
            fc1_mm(1, 1)
            for g in range(1, ngroup):
                sig_b(g)
                if g == 1:
                    fc1_mm(2, 0)
                    fc1_mm(2, 1)
                sig_c(g - 1)
            fc1_mm(3, 0)
            fc1_mm(3, 1)
            sig_c(ngroup - 1)

            # ======================= MLP =======================
            # fc1: H1[b, h] = FT.T @ W1T, consuming chunks as they arrive
            h1 = actp.tile([BPC, HID], f32)
            h1t = actp.tile([128, 128], bf16)
            # all but the last chunk: both halves interleaved; last chunk
            # finishes half 0 first so its relu/transposes overlap half 1
            for kt in range(4, NT1 - 4):
                fc1_mm(kt, 0)
                fc1_mm(kt, 1)
            for nt in range(2):
                for kt in range(NT1 - 4, NT1):
                    fc1_mm(kt, nt)

            # fc2, interleaved with the two h1 transpose halves; biases
            # open the accumulation so nothing trails it
            h2ps = [mlpps.tile([BPC, 512], f32, tag="hps", name=f"h2ps{i}")
                    for i in range(2)]
            def transp_half(src, dst, half):
                tps = []
                for i in range(4 * half, 4 * half + 4):
                    tp = smallps.tile([128, 65], f32, tag="sm", name="tp")
                    nc.tensor.transpose(
                        tp[:, 0:BPC], src[:, 128 * i:128 * i + 128],
                        idn16[0:BPC, 0:BPC])
                    tps.append((i, tp))
                for i, tp in tps:
                    nc.vector.tensor_copy(
                        dst[:, 16 * i:16 * i + 16], tp[:, 0:BPC])

            def fc2_mm(kt, nt, start=False, stop=False):
                nc.tensor.matmul(
                    h2ps[nt][:, :],
                    h1t[:, 16 * kt:16 * kt + 16],
                    w2tiles[kt // 4][:, HID * (kt % 4) + 512 * nt:
                                     HID * (kt % 4) + 512 * nt + 512],
                    start=start, stop=stop)

            # half 0 pieces pipeline relu -> transpose -> copy -> matmul
            tps = []
            for i in range(4):
                nc.vector.tensor_relu(h1[:, 128 * i:128 * i + 128],
                                      h1ps[0][:, 128 * i:128 * i + 128])
                tp = smallps.tile([128, 65], f32, tag="sm", name="tp")
                nc.tensor.transpose(
                    tp[:, 0:BPC], h1[:, 128 * i:128 * i + 128],
                    idn16[0:BPC, 0:BPC])
                tps.append(tp)
                nc.vector.tensor_copy(
                    h1t[:, 16 * i:16 * i + 16], tp[:, 0:BPC])
            for kt in range(0, 4):
                fc2_mm(kt, 0, start=(kt == 0))
            # half 1 hides under the half-0 fc2 matmuls; the nt=1
            # accumulation only opens after relu1 frees h1ps[1]'s bank
            nc.vector.tensor_relu(h1[:, 512:1024], h1ps[1][:, :])
            transp_half(h1, h1t, 1)
            for kt in range(0, 4):
                fc2_mm(kt, 1, start=(kt == 0))
            for kt in range(4, 8):
                fc2_mm(kt, 0)
            nc.tensor.matmul(h2ps[0][:, :], ones16[:, :], b2s[0:1, 0:512],
                             start=False, stop=True)
            for kt in range(4, 8):
                fc2_mm(kt, 1)
            nc.tensor.matmul(h2ps[1][:, :], ones16[:, :], b2s[0:1, 512:1024],
                             start=False, stop=True)

            # fc3, interleaved with the two h2 transpose halves
            h2 = actp.tile([BPC, HID], f32)
            h2t = actp.tile([128, 128], bf16)
            ops = mlpps.tile([BPC, 512], f32, tag="hps")
            for half in range(2):
                nc.vector.tensor_relu(h2[:, 512 * half:512 * half + 512],
                                      h2ps[half][:, :])
                transp_half(h2, h2t, half)
                for kt in range(4 * half, 4 * half + 4):
                    nc.tensor.matmul(
                        ops[:, 0:OUT_DIM],
                        h2t[:, 16 * kt:16 * kt + 16],
                        w3s[:, OUT_DIM * kt:OUT_DIM * kt + OUT_DIM],
                        start=(kt == 0), stop=False)
            nc.tensor.matmul(
                ops[:, 0:OUT_DIM], ones16[:, :], b3s, start=False, stop=True)
            outsb = actp.tile([BPC, OUT_DIM], f32)
            nc.vector.tensor_copy(outsb[:, :], ops[:, 0:OUT_DIM])
            nc.sync.dma_start(outa[:, :], outsb[:, :])

    nc.compile()
    _prog_cache[key] = nc
    return nc


def _host_weights(conv_w, conv_b, fc1_w, fc1_b, fc2_w, fc2_b, fc3_w, fc3_b):
    import ml_dtypes
    f = np.float32
    bf = ml_dtypes.bfloat16
    conv_w = np.asarray(conv_w, f)
    fc1_w = np.asarray(fc1_w, f)

    wkb = np.zeros((128, 480), f)
    # conv taps: wcv[i, 64k + o] = conv_w[o, i, k]; row 64 = bias (tap 0)
    wt = conv_w.transpose(1, 2, 0)                 # [i, k, o]
    wkb[0:64, 0:192] = wt.reshape(64, 192)
    wkb[64, 0:64] = np.asarray(conv_b, f)
    # ocst ones-columns: per (elem-in-group, block); zero at (127, blk 3)
    wkb[:, 192:208] = 1.0
    wkb[127, 192 + 3::4] = 0.0
    wkb[0, 208:224] = 1.0                          # ones16 row
    for m in range(127):
        wkb[m + 1, 224 + m] = 1.0                  # shift-by-one identity
    wkb[0, 352 + 127] = 1.0                        # e127 row

    # fc1 signature weights consume T = st - st^T (device feature
    # F(a,b) = T[a,b]): word (i<j) lives on F(j,i) with +w/2; when both
    # orders of a pair are present in the packed set each carries +-w/4
    wtri = fc1_w[:, 64:2080]
    iu, ju = np.triu_indices(64, 1)
    wid = np.zeros((64, 64), np.int32)
    wid[iu, ju] = np.arange(2016)

    feats = []                           # (a, b) per packed row, tile order
    for tau in range(16):
        for p in range(128):
            feats.append((p, tau) if p < 64 else (p - 64, 16 + tau))
    for tau in range(8):
        for p in range(128):
            sl, q = divmod(p, 32)
            feats.append((32 + q, 32 + 8 * sl + tau))

    def present(a, b):
        return (b < 32) or (a >= 32 and b >= 32)

    w1t = np.zeros((D1, HID), f)
    # rows 0:512 = aux K-tiles in stream order [t25, t26, t27, t24]
    w1t[0:128, :] = fc1_w[:, 2145:2273].T          # static chans 129..256
    w1t[128:256, :] = fc1_w[:, 2273:2401].T        # static chans 257..384
    w1t[256, :] = fc1_w[:, 2080]                   # pooled
    w1t[257, :] = np.asarray(fc1_b, f)             # const-1 -> fc1 bias
    w1t[258:321, :] = fc1_w[:, 2401:2464].T        # static chans 385..447
    w1t[384:448, :] = fc1_w[:, 0:64].T             # lvl1
    w1t[448:512, :] = fc1_w[:, 2081:2145].T        # static chans 65..128
    for n, (a, b) in enumerate(feats):
        if a == b:
            continue
        i, j = (a, b) if a < b else (b, a)
        sgn = 1.0 if a > b else -1.0
        mult = 2.0 if (present(a, b) and present(b, a)) else 1.0
        w1t[512 + n, :] = (sgn * 0.5 / mult) * wtri[:, wid[i, j]]

    w2t = np.ascontiguousarray(np.asarray(fc2_w, f).T)
    # w3 pre-packed into the on-chip layout [128, kt*128 + o]
    w3t = np.asarray(fc3_w, f).T.reshape(8, 128, OUT_DIM)
    w3t = np.ascontiguousarray(w3t.transpose(1, 0, 2).reshape(128, HID))
    b23 = np.concatenate(
        [np.asarray(fc2_b, f), np.asarray(fc3_b, f)])[None, :]
    return dict(wkb=wkb.astype(bf), b23=b23.astype(bf),
                w1t=w1t.astype(bf), w2t=w2t.astype(bf), w3t=w3t.astype(bf))


def make_in_maps(x, conv_w, conv_b, fc1_w, fc1_b, fc2_w, fc2_b, fc3_w, fc3_b):
    import ml_dtypes
    bf = ml_dtypes.bfloat16
    wkey = id(fc1_w)
    if _host_cache.get("wkey") != wkey:
        _host_cache["shared"] = _host_weights(
            conv_w, conv_b, fc1_w, fc1_b, fc2_w, fc2_b, fc3_w, fc3_b)
        _host_cache["wkey"] = wkey
    shared = _host_cache["shared"]
    x = np.asarray(x, np.float32)
    eye16 = np.eye(BPC, dtype=np.float32)
    in_maps = []
    for c in range(NCORES):
        xc = x[BPC * c:BPC * (c + 1)]
        m = dict(shared)
        xcv = np.zeros((BPC, C_IN + 1, XW), np.float32)
        xcv[:, 0:C_IN, 1:513] = xc[:, 0:C_IN, :]
        xcv[:, C_IN, :] = 1.0                            # conv bias ones row
        m["xcv"] = xcv.astype(bf)
        m["xpi"] = np.concatenate([xc[:, C_IN, :], eye16], axis=1)
        m["c64"] = np.eye(64, dtype=np.float32)
        aux = np.zeros((128, 4 * BPC), np.float32)
        aux[64:128, 0:BPC] = xc[:, 65:129, 0].T          # tile 32 (p>=64)
        aux[:, BPC:2 * BPC] = xc[:, 129:257, 0].T        # tile 33
        aux[:, 2 * BPC:3 * BPC] = xc[:, 257:385, 0].T    # tile 34
        aux[1, 3 * BPC:4 * BPC] = 1.0                    # fc1 bias row
        aux[2:65, 3 * BPC:4 * BPC] = xc[:, 385:448, 0].T  # tile 35 statics
        m["aux"] = aux.astype(bf)
        in_maps.append(m)
    return in_maps


def kernel(x, conv_w, conv_b, fc1_w, fc1_b, fc2_w, fc2_b, fc3_w, fc3_b):
    from concourse.bass_utils import run_bass_kernel_spmd

    nc = _build_nc()
    in_maps = make_in_maps(x, conv_w, conv_b, fc1_w, fc1_b, fc2_w, fc2_b,
                           fc3_w, fc3_b)
    res = run_bass_kernel_spmd(nc, in_maps, list(range(NCORES)))
    out = np.concatenate([res.results[c]["out"] for c in range(NCORES)], axis=0)
    return out.astype(np.float32)
